# revision 57
# baseline (speedup 1.0000x reference)
"""Trainium2 Bass kernel for nn_ComparisonLayer.

Math (reference):
    x: [L=512, B=2, D=256] -> transpose to [B, L, D], layernorm over D
    a = xn @ w1.T + b1                  # [B, L, C=128]
    b = xn @ w2.T + b2                  # [B, L, C]
    out[b,i,j,o] = sum_c a[b,i,c]*b[b,j,c]*w3[o,c] + b3[o]
                 + sum_c (a[b,i,c]-b[b,j,c])*w4[o,c]      # [B, L, L, O=64]

Decomposition (device does the O(L^2) work; host does the O(L) input prep):
    out[b,i,j,o] = sum_c a[b,i,c]*b[b,j,c]*w3[o,c]        # MM_A, K=128
                 + A4[b,i,o] + Bterm[b,j,o]               # MM_B, K=65
    A4 = a @ w4.T;  Bterm = b3 - b @ w4.T
  - Host (numpy, f64): layernorm + the input GEMMs -> a, b, A4, Bterm. This
    matches the sharding hint's starting point ("a sliced / b replicated ...
    fused GEMMs"); >98% of FLOPs (the L*L*C contraction) stay on device.
  - Device per batch: V3[c,(j,o)] = bT[c,j]*w3T[c,o] (DVE/Pool elementwise),
    then per 512-wide (j,o) section two accumulating matmuls:
        psum  = Aug.T @ R        K=65: row of ones x Bterm[(j,o)] +
                                 A4T[o',i] x rid[o',(j,o)]  (rid = delta(o'=o))
        psum += aT_it.T @ V3     K=128 main contraction, fp16 inputs
    A chain of tiny K=1 matmuls warms the PE p-state while inputs stream in.
    Epilogue casts psum (f32) -> fp16 stage (ACT/DVE split 20/12); every
    1024-column quarter is stored with its own 256KB DMA so the store queue
    starts draining as soon as the first quarter exists.

Sharding: second L (the j axis) split across the 8 cores; each core gets the
full aT/A4 plus its own 64-row slice of b and returns out[:, :, 64k:64k+64, :]
in fp16; the host concatenates along axis 2 and upcasts to f32.
"""

import sys

if "/opt/trn_rl_repo" not in sys.path:
    sys.path.insert(0, "/opt/trn_rl_repo")

from contextlib import ExitStack

import numpy as np

import concourse.bacc as bacc
import concourse.mybir as mybir
import concourse.tile as tile
from concourse.alu_op_type import AluOpType
from concourse.bass_utils import run_bass_kernel_spmd

L, B, D = 512, 2, 256
C, O = 128, 64
NCORES = 8
JS = L // NCORES  # 64 j's per core
JB = 8  # j's per 512-wide section
F32 = mybir.dt.float32
FP16 = mybir.dt.float16
ACT_COPY = mybir.ActivationFunctionType.Copy


def build_nc(niter=1):
    nc = bacc.Bacc("TRN2", target_bir_lowering=False)

    # atw: [c, bT(b0) | bT(b1) | w3T | aT(b*L+i)]                  fp16
    # ra0: [65, ones|a4T (1024) | b0: Bterm|rid (4096)]            fp16
    # ra1: [1, b1: Bterm (4096)] (rid rows are rebuilt on-device)  fp16
    # (row 0 carries Bterm / the ones row; rows 1..64 carry
    #  rid[o',(j,o)] = delta(o'==o) / a4T[o', b*L+i])
    atw = nc.dram_tensor("atw", [C, B * L + 3 * O], FP16, kind="ExternalInput")
    ra0 = nc.dram_tensor("ra0", [O + 1, JS * O + B * L], FP16,
                         kind="ExternalInput")
    ra1 = nc.dram_tensor("ra1", [1, JS * O], FP16, kind="ExternalInput")
    out = nc.dram_tensor("out", [B, L, JS, O], FP16, kind="ExternalOutput")

    NSEC = JS * O // 512  # 8 sections of 512 per (b, i-tile)

    with tile.TileContext(nc) as tc:
        for rep in range(niter):
          with ExitStack() as ctx:
            consts = ctx.enter_context(tc.tile_pool(name=f"consts{rep}", bufs=1))
            big = ctx.enter_context(tc.tile_pool(name=f"big{rep}", bufs=1))
            ps_pool = ctx.enter_context(
                tc.tile_pool(name=f"ps{rep}", bufs=4, space="PSUM"))
            stage_pool = ctx.enter_context(
                tc.tile_pool(name=f"stage{rep}", bufs=4))

            # ---- PE p-state warmup: a chain of tiny K=1 matmuls keeps the
            # tensor engine busy while the inputs stream in, so the real
            # matmuls start at full clock instead of re-ramping.
            ones1 = consts.tile([1, C], FP16)
            nc.vector.memset(ones1, 1.0)
            wps = ps_pool.tile([128, 1024], F32, tag="ps_main", name="ps_warm")
            for _ in range(62):
                nc.tensor.matmul(out=wps[0:64, 0:64], lhsT=ones1[:, 0:64],
                                 rhs=ones1[:, 0:64], start=True, stop=True)

            # ---- ACT table warmup (overlaps the input DMAs) ----
            warm = consts.tile([1, 8], F32)
            nc.vector.memset(warm, 1.0)
            nc.scalar.activation(out=warm, in_=warm, func=ACT_COPY)

            # ---- input loads: one queue, in dependency-priority order ----
            atw_sb = consts.tile([C, B * L + 3 * O], FP16)
            ra0_sb = consts.tile([O + 1, JS * O + B * L], FP16)
            ra1_sb = consts.tile([O + 1, JS * O], FP16)
            # W + it0's aT slice in a tiny first DMA (feeds V3 + first MM_As),
            # then ra0 in slices (feeds the first MM_Bs asap), then the rest.
            nc.sync.dma_start(out=atw_sb[:, 0:3 * O + 128],
                              in_=atw.ap()[:, 0:3 * O + 128])
            nc.sync.dma_start(out=ra0_sb[:, 0:2048], in_=ra0.ap()[:, 0:2048])
            nc.sync.dma_start(out=ra0_sb[:, 2048:3072],
                              in_=ra0.ap()[:, 2048:3072])
            nc.sync.dma_start(out=atw_sb[:, 3 * O + 128:],
                              in_=atw.ap()[:, 3 * O + 128:])
            nc.sync.dma_start(out=ra0_sb[:, 3072:], in_=ra0.ap()[:, 3072:])
            bT = [atw_sb[:, 0:O], atw_sb[:, O:2 * O]]
            w3T = atw_sb[:, 2 * O:3 * O]
            aT_sb = atw_sb[:, 3 * O:3 * O + B * L]
            Aug = ra0_sb[:, 0:B * L]
            Rb = [ra0_sb[:, B * L:B * L + JS * O], ra1_sb]

            # ---- V3[b][c, (j,o)] = bT[b][c,j]*w3T[c,o] ----
            V3 = [big.tile([C, JS * O], FP16, name=f"r{rep}_V3{b_}")
                  for b_ in range(B)]
            w3b = w3T.unsqueeze(1).broadcast_to([C, JB, O])

            def emit_v3(bb, jb):
                # b0 chunks on DVE (fast, feed the first matmul sweep); b1
                # chunks on the otherwise-idle Pool, done well before b1 runs.
                sl = slice(jb * JB, (jb + 1) * JB)
                v = V3[bb].rearrange("c (j o) -> c j o", j=JS)[:, sl, :]
                bT3 = bT[bb][:, sl].unsqueeze(2).broadcast_to([C, JB, O])
                eng = nc.vector if bb == 0 else nc.gpsimd
                eng.tensor_tensor(out=v, in0=bT3, in1=w3b, op=AluOpType.mult)

            for jb in range(NSEC):
                emit_v3(0, jb)
            # b1's rid block (identical to b0's) is copied SBUF->SBUF on the
            # DVE instead of paying a second 524KB trip through the DMA
            # engines; its Bterm row then lands on top via a tiny DMA.
            nc.vector.tensor_copy(out=ra1_sb, in_=Rb[0])
            nc.sync.dma_start(out=ra1_sb[0:1, :], in_=ra1.ap())
            for jb in range(NSEC):
                emit_v3(1, jb)

            # ---- main loop ----
            nepi = 0
            for bb in range(B):
                for it in range(4):
                    lhs_a = aT_sb[:, bb * L + it * 128: bb * L + (it + 1) * 128]
                    lhs_g = Aug[:, bb * L + it * 128: bb * L + (it + 1) * 128]
                    stage = stage_pool.tile([128, JS * O], FP16, tag="stage")
                    # Per 2-psum-tile group: MM_Bs first (they only need ra0,
                    # which lands before V3 is ready), then the MM_As.
                    for grp in (range(2), range(2, 4)):
                        pss = {}
                        for t in grp:
                            pss[t] = ps_pool.tile(
                                [128, 1024], F32, tag="ps_main",
                                name=f"ps_{bb}_{it}_{t}")
                            for sec in range(2):
                                col0 = (t * 2 + sec) * 512
                                nc.tensor.matmul(
                                    out=pss[t][:, sec * 512:(sec + 1) * 512],
                                    lhsT=lhs_g,
                                    rhs=Rb[bb][:, col0:col0 + 512],
                                    start=True, stop=False)
                        for t in grp:
                            for sec in range(2):
                                col0 = (t * 2 + sec) * 512
                                nc.tensor.matmul(
                                    out=pss[t][:, sec * 512:(sec + 1) * 512],
                                    lhsT=lhs_a,
                                    rhs=V3[bb][:, col0:col0 + 512],
                                    start=False, stop=True)
                        for t in grp:
                            dst = stage[:, t * 1024:(t + 1) * 1024]
                            # 20 ACT / 12 DVE epilogue split (ACT is faster
                            # per op and DVE also carries V3 mult chunks).
                            if nepi % 8 in (2, 5, 7):
                                nc.vector.tensor_copy(out=dst, in_=pss[t])
                            else:
                                nc.scalar.activation(out=dst, in_=pss[t],
                                                     func=ACT_COPY)
                            nepi += 1
                            if nepi == 31:
                                continue  # merged into the final half-store
                            if nepi == 32:
                                # final group: one half-store (fewer issue
                                # slots in the congested last window)
                                nc.sync.dma_start(
                                    out=out.ap()[bb, it * 128:(it + 1) * 128,
                                                 32:64, :],
                                    in_=stage[:, 2048:4096]
                                    .rearrange("p (j o) -> p j o", j=32))
                                continue
                            # store per quarter: the first stores start as
                            # early as possible (no DMA idle window) and the
                            # final quarters drain with minimum tail
                            nc.sync.dma_start(
                                out=out.ap()[bb, it * 128:(it + 1) * 128,
                                             t * 16:(t + 1) * 16, :],
                                in_=stage[:, t * 1024:(t + 1) * 1024]
                                .rearrange("p (j o) -> p j o", j=16))

    nc.compile()
    return nc


_NC = None


def _host_prep(inputs):
    """Exact reference input-side math in f64: layernorm + a/b GEMMs."""
    f64 = lambda v: np.asarray(v, dtype=np.float64)
    x = f64(inputs["x"]).transpose(1, 0, 2)  # [B, L, D]
    mu = x.mean(axis=-1, keepdims=True)
    var = x.var(axis=-1, keepdims=True)
    xn = (x - mu) / np.sqrt(var + 1e-5) * f64(inputs["norm_w"]) + f64(
        inputs["norm_b"])
    a = xn @ f64(inputs["w1"]).T + f64(inputs["b1"])  # [B, L, C]
    b = xn @ f64(inputs["w2"]).T + f64(inputs["b2"])  # [B, L, C]
    a4 = a @ f64(inputs["w4"]).T                      # [B, L, O]
    bterm = f64(inputs["b3"])[None, None, :] - b @ f64(inputs["w4"]).T
    return a, b, a4, bterm


def kernel(**inputs):
    global _NC
    if _NC is None:
        _NC = build_nc()
    a, b, a4, bterm = _host_prep(inputs)
    w3T = np.asarray(inputs["w3"], np.float64).T  # [C, O]
    aT_np = np.concatenate([a[0].T, a[1].T], axis=1)
    a4T_np = np.concatenate([a4[0].T, a4[1].T], axis=1)  # [O, B*L]
    rid_np = np.tile(np.eye(O), (1, JS))  # [O, JS*O]
    in_maps = []
    for k in range(NCORES):
        jsl = slice(k * JS, (k + 1) * JS)
        atw_np = np.concatenate(
            [b[0, jsl].T, b[1, jsl].T, w3T, aT_np], axis=1).astype(np.float16)
        ra0_np = np.concatenate(
            [np.concatenate([np.ones((1, B * L)), a4T_np], axis=0),
             np.concatenate(
                 [bterm[0, jsl].reshape(1, JS * O), rid_np], axis=0)],
            axis=1).astype(np.float16)
        in_maps.append({
            "atw": np.ascontiguousarray(atw_np),
            "ra0": np.ascontiguousarray(ra0_np),
            # ra1 ships only b1's Bterm row; the rid rows are copied from
            # ra0 on-device
            "ra1": np.ascontiguousarray(
                bterm[1, jsl].reshape(1, JS * O).astype(np.float16)),
        })
    # The axon-tunneled device occasionally reports a transient
    # "unrecoverable" state from a previous session; a short backoff and
    # retry recovers it.
    last_err = None
    for attempt in range(3):
        try:
            res = run_bass_kernel_spmd(_NC, in_maps, core_ids=list(range(NCORES)))
            break
        except Exception as e:
            last_err = e
            if attempt == 2:
                raise
            import time as _time
            _time.sleep(45)
    shards = [res.results[k]["out"].astype(np.float32) for k in range(NCORES)]
    return np.concatenate(shards, axis=2)


# revision 58
# speedup vs baseline: 1.0165x; 1.0165x over previous
"""Trainium2 Bass kernel for nn_ComparisonLayer.

Math (reference):
    x: [L=512, B=2, D=256] -> transpose to [B, L, D], layernorm over D
    a = xn @ w1.T + b1                  # [B, L, C=128]
    b = xn @ w2.T + b2                  # [B, L, C]
    out[b,i,j,o] = sum_c a[b,i,c]*b[b,j,c]*w3[o,c] + b3[o]
                 + sum_c (a[b,i,c]-b[b,j,c])*w4[o,c]      # [B, L, L, O=64]

Decomposition (device does the O(L^2) work; host does the O(L) input prep):
    out[b,i,j,o] = sum_c a[b,i,c]*b[b,j,c]*w3[o,c]        # MM_A, K=128
                 + A4[b,i,o] + Bterm[b,j,o]               # MM_B, K=65
    A4 = a @ w4.T;  Bterm = b3 - b @ w4.T
  - Host (numpy, f64): layernorm + the input GEMMs -> a, b, A4, Bterm. This
    matches the sharding hint's starting point ("a sliced / b replicated ...
    fused GEMMs"); >98% of FLOPs (the L*L*C contraction) stay on device.
  - Device per batch: V3[c,(j,o)] = bT[c,j]*w3T[c,o] (DVE/Pool elementwise),
    then per 512-wide (j,o) section two accumulating matmuls:
        psum  = Aug.T @ R        K=65: row of ones x Bterm[(j,o)] +
                                 A4T[o',i] x rid[o',(j,o)]  (rid = delta(o'=o))
        psum += aT_it.T @ V3     K=128 main contraction, fp16 inputs
    A chain of tiny K=1 matmuls warms the PE p-state while inputs stream in.
    Epilogue casts psum (f32) -> fp16 stage (ACT/DVE split 20/12); every
    1024-column quarter is stored with its own 256KB DMA so the store queue
    starts draining as soon as the first quarter exists.

Sharding: second L (the j axis) split across the 8 cores; each core gets the
full aT/A4 plus its own 64-row slice of b and returns out[:, :, 64k:64k+64, :]
in fp16; the host concatenates along axis 2 and upcasts to f32.
"""

import sys

if "/opt/trn_rl_repo" not in sys.path:
    sys.path.insert(0, "/opt/trn_rl_repo")

from contextlib import ExitStack

import numpy as np

import concourse.bacc as bacc
import concourse.mybir as mybir
import concourse.tile as tile
from concourse.alu_op_type import AluOpType
from concourse.bass_utils import run_bass_kernel_spmd

L, B, D = 512, 2, 256
C, O = 128, 64
NCORES = 8
JS = L // NCORES  # 64 j's per core
JB = 8  # j's per 512-wide section
F32 = mybir.dt.float32
FP16 = mybir.dt.float16
ACT_COPY = mybir.ActivationFunctionType.Copy


def build_nc(niter=1):
    nc = bacc.Bacc("TRN2", target_bir_lowering=False)

    # atw: [c, bT(b0) | bT(b1) | w3T | aT(b*L+i)]                  fp16
    # ra0: [65, ones|a4T (1024) | b0: Bterm|rid (4096)]            fp16
    # ra1: [1, b1: Bterm (4096)] (rid rows are rebuilt on-device)  fp16
    # (row 0 carries Bterm / the ones row; rows 1..64 carry
    #  rid[o',(j,o)] = delta(o'==o) / a4T[o', b*L+i])
    atw = nc.dram_tensor("atw", [C, B * L + 3 * O], FP16, kind="ExternalInput")
    ra0 = nc.dram_tensor("ra0", [O + 1, JS * O + B * L], FP16,
                         kind="ExternalInput")
    ra1 = nc.dram_tensor("ra1", [1, JS * O], FP16, kind="ExternalInput")
    out = nc.dram_tensor("out", [B, L, JS, O], FP16, kind="ExternalOutput")

    NSEC = JS * O // 512  # 8 sections of 512 per (b, i-tile)

    with tile.TileContext(nc) as tc:
        for rep in range(niter):
          with ExitStack() as ctx:
            consts = ctx.enter_context(tc.tile_pool(name=f"consts{rep}", bufs=1))
            big = ctx.enter_context(tc.tile_pool(name=f"big{rep}", bufs=1))
            ps_pool = ctx.enter_context(
                tc.tile_pool(name=f"ps{rep}", bufs=4, space="PSUM"))
            stage_pool = ctx.enter_context(
                tc.tile_pool(name=f"stage{rep}", bufs=4))

            # ---- PE p-state warmup: a chain of tiny K=1 matmuls keeps the
            # tensor engine busy while the inputs stream in, so the real
            # matmuls start at full clock instead of re-ramping.
            ones1 = consts.tile([1, C], FP16)
            nc.vector.memset(ones1, 1.0)
            wps = ps_pool.tile([128, 1024], F32, tag="ps_main", name="ps_warm")
            for _ in range(62):
                nc.tensor.matmul(out=wps[0:64, 0:64], lhsT=ones1[:, 0:64],
                                 rhs=ones1[:, 0:64], start=True, stop=True)

            # ---- ACT table warmup (overlaps the input DMAs) ----
            warm = consts.tile([1, 8], F32)
            nc.vector.memset(warm, 1.0)
            nc.scalar.activation(out=warm, in_=warm, func=ACT_COPY)

            # ---- input loads: one queue, in dependency-priority order ----
            atw_sb = consts.tile([C, B * L + 3 * O], FP16)
            ra0_sb = consts.tile([O + 1, JS * O + B * L], FP16)
            ra1_sb = consts.tile([O + 1, JS * O], FP16)
            # W + it0's aT slice in a tiny first DMA (feeds V3 + first MM_As),
            # then ra0 in slices (feeds the first MM_Bs asap), then the rest.
            nc.sync.dma_start(out=atw_sb[:, 0:3 * O + 128],
                              in_=atw.ap()[:, 0:3 * O + 128])
            nc.sync.dma_start(out=ra0_sb[:, 0:2048], in_=ra0.ap()[:, 0:2048])
            nc.sync.dma_start(out=ra0_sb[:, 2048:3072],
                              in_=ra0.ap()[:, 2048:3072])
            nc.sync.dma_start(out=atw_sb[:, 3 * O + 128:],
                              in_=atw.ap()[:, 3 * O + 128:])
            nc.sync.dma_start(out=ra0_sb[:, 3072:], in_=ra0.ap()[:, 3072:])
            bT = [atw_sb[:, 0:O], atw_sb[:, O:2 * O]]
            w3T = atw_sb[:, 2 * O:3 * O]
            aT_sb = atw_sb[:, 3 * O:3 * O + B * L]
            Aug = ra0_sb[:, 0:B * L]
            Rb = [ra0_sb[:, B * L:B * L + JS * O], ra1_sb]

            # ---- V3[b][c, (j,o)] = bT[b][c,j]*w3T[c,o] ----
            V3 = [big.tile([C, JS * O], FP16, name=f"r{rep}_V3{b_}")
                  for b_ in range(B)]
            w3b = w3T.unsqueeze(1).broadcast_to([C, JB, O])

            def emit_v3(bb, jb):
                # b0 chunks on DVE (fast, feed the first matmul sweep); b1
                # chunks on the otherwise-idle Pool, done well before b1 runs.
                sl = slice(jb * JB, (jb + 1) * JB)
                v = V3[bb].rearrange("c (j o) -> c j o", j=JS)[:, sl, :]
                bT3 = bT[bb][:, sl].unsqueeze(2).broadcast_to([C, JB, O])
                eng = nc.vector if bb == 0 else nc.gpsimd
                eng.tensor_tensor(out=v, in0=bT3, in1=w3b, op=AluOpType.mult)

            for jb in range(NSEC):
                emit_v3(0, jb)
            # b1's rid block (identical to b0's) is copied SBUF->SBUF on the
            # DVE instead of paying a second 524KB trip through the DMA
            # engines; its Bterm row then lands on top via a tiny DMA.
            nc.vector.tensor_copy(out=ra1_sb, in_=Rb[0])
            nc.sync.dma_start(out=ra1_sb[0:1, :], in_=ra1.ap())
            for jb in range(NSEC):
                emit_v3(1, jb)

            # ---- main loop ----
            nepi = 0
            for bb in range(B):
                for it in range(4):
                    lhs_a = aT_sb[:, bb * L + it * 128: bb * L + (it + 1) * 128]
                    lhs_g = Aug[:, bb * L + it * 128: bb * L + (it + 1) * 128]
                    stage = stage_pool.tile([128, JS * O], FP16, tag="stage")
                    # Per 2-psum-tile group: MM_Bs first (they only need ra0,
                    # which lands before V3 is ready), then the MM_As.
                    for grp in (range(2), range(2, 4)):
                        pss = {}
                        for t in grp:
                            pss[t] = ps_pool.tile(
                                [128, 1024], F32, tag="ps_main",
                                name=f"ps_{bb}_{it}_{t}")
                            for sec in range(2):
                                col0 = (t * 2 + sec) * 512
                                nc.tensor.matmul(
                                    out=pss[t][:, sec * 512:(sec + 1) * 512],
                                    lhsT=lhs_g,
                                    rhs=Rb[bb][:, col0:col0 + 512],
                                    start=True, stop=False)
                        for t in grp:
                            for sec in range(2):
                                col0 = (t * 2 + sec) * 512
                                nc.tensor.matmul(
                                    out=pss[t][:, sec * 512:(sec + 1) * 512],
                                    lhsT=lhs_a,
                                    rhs=V3[bb][:, col0:col0 + 512],
                                    start=False, stop=True)
                        for t in grp:
                            dst = stage[:, t * 1024:(t + 1) * 1024]
                            # 20 ACT / 12 DVE epilogue split (ACT is faster
                            # per op and DVE also carries V3 mult chunks).
                            if nepi % 8 in (2, 5, 7):
                                nc.vector.tensor_copy(out=dst, in_=pss[t])
                            else:
                                nc.scalar.activation(out=dst, in_=pss[t],
                                                     func=ACT_COPY)
                            nepi += 1
                            # store per quarter: the first stores start as
                            # early as possible (no DMA idle window) and the
                            # final quarters drain with minimum tail
                            nc.sync.dma_start(
                                out=out.ap()[bb, it * 128:(it + 1) * 128,
                                             t * 16:(t + 1) * 16, :],
                                in_=stage[:, t * 1024:(t + 1) * 1024]
                                .rearrange("p (j o) -> p j o", j=16))

    nc.compile()
    return nc


_NC = None


def _host_prep(inputs):
    """Exact reference input-side math in f64: layernorm + a/b GEMMs."""
    f64 = lambda v: np.asarray(v, dtype=np.float64)
    x = f64(inputs["x"]).transpose(1, 0, 2)  # [B, L, D]
    mu = x.mean(axis=-1, keepdims=True)
    var = x.var(axis=-1, keepdims=True)
    xn = (x - mu) / np.sqrt(var + 1e-5) * f64(inputs["norm_w"]) + f64(
        inputs["norm_b"])
    a = xn @ f64(inputs["w1"]).T + f64(inputs["b1"])  # [B, L, C]
    b = xn @ f64(inputs["w2"]).T + f64(inputs["b2"])  # [B, L, C]
    a4 = a @ f64(inputs["w4"]).T                      # [B, L, O]
    bterm = f64(inputs["b3"])[None, None, :] - b @ f64(inputs["w4"]).T
    return a, b, a4, bterm


def kernel(**inputs):
    global _NC
    if _NC is None:
        _NC = build_nc()
    a, b, a4, bterm = _host_prep(inputs)
    w3T = np.asarray(inputs["w3"], np.float64).T  # [C, O]
    aT_np = np.concatenate([a[0].T, a[1].T], axis=1)
    a4T_np = np.concatenate([a4[0].T, a4[1].T], axis=1)  # [O, B*L]
    rid_np = np.tile(np.eye(O), (1, JS))  # [O, JS*O]
    in_maps = []
    for k in range(NCORES):
        jsl = slice(k * JS, (k + 1) * JS)
        atw_np = np.concatenate(
            [b[0, jsl].T, b[1, jsl].T, w3T, aT_np], axis=1).astype(np.float16)
        ra0_np = np.concatenate(
            [np.concatenate([np.ones((1, B * L)), a4T_np], axis=0),
             np.concatenate(
                 [bterm[0, jsl].reshape(1, JS * O), rid_np], axis=0)],
            axis=1).astype(np.float16)
        in_maps.append({
            "atw": np.ascontiguousarray(atw_np),
            "ra0": np.ascontiguousarray(ra0_np),
            # ra1 ships only b1's Bterm row; the rid rows are copied from
            # ra0 on-device
            "ra1": np.ascontiguousarray(
                bterm[1, jsl].reshape(1, JS * O).astype(np.float16)),
        })
    # The axon-tunneled device occasionally reports a transient
    # "unrecoverable" state from a previous session; a short backoff and
    # retry recovers it.
    last_err = None
    for attempt in range(3):
        try:
            res = run_bass_kernel_spmd(_NC, in_maps, core_ids=list(range(NCORES)))
            break
        except Exception as e:
            last_err = e
            if attempt == 2:
                raise
            import time as _time
            _time.sleep(45)
    shards = [res.results[k]["out"].astype(np.float32) for k in range(NCORES)]
    return np.concatenate(shards, axis=2)
